# revision 1
# baseline (speedup 1.0000x reference)
"""ComplEx scoring kernel for Trainium2, sharded over 8 NeuronCores.

Computes: result[b, e] = tmp1[b] . E_im[e] + tmp2[b] . E_re[e] + mask[e]
where tmp1/tmp2 are complex-product combinations of gathered entity and
relation embeddings (with inverse-relation sign handling).

Sharding: entity dimension (100000) split across 8 cores (12500 each).
Batch and relation tables replicated. Each core redundantly computes the
gathered tmp1/tmp2 (needs the full entity table for the gather), then
GEMMs against its own entity shard and emits [1024, 12500] logits.
"""

import sys

sys.path.insert(0, "/opt/trn_rl_repo")

import numpy as np

import concourse.bacc as bacc
import concourse.bass as bass
import concourse.mybir as mybir
import concourse.tile as tile
from concourse.bass import IndirectOffsetOnAxis
from concourse.bass_utils import run_bass_kernel_spmd
from concourse.masks import make_identity

F32 = mybir.dt.float32
F32R = mybir.dt.float32r
I32 = mybir.dt.int32

NUM_ENTITIES = 100000
DIM = 512
BATCH = 1024
NUM_REL = 500  # NUM_REL_TOTAL // 2
N_CORES = 8
ESH = NUM_ENTITIES // N_CORES  # 12500 entities per core
ET = 500                       # entity tile (free dim of each matmul)
N_ET = ESH // ET               # 25 e-tiles per core
NB = BATCH // 128              # 8 batch tiles
NC_D = DIM // 128              # 4 contraction chunks per table


def build_module(
    use_f32r=True,
    nrep=1,
    do_prologue=True,   # gathers + elementwise + transposes (else memset tmpT)
    do_rhs_dma=True,    # stream rhs tiles from DRAM (else one static tile)
    do_mm=True,         # matmuls
    store_all=True,     # all output stores (else only et==0)
):
    nc = bacc.Bacc("TRN2", target_bir_lowering=False, debug=False)

    hix = nc.dram_tensor("hix", [128, NB], I32, kind="ExternalInput")
    rix = nc.dram_tensor("rix", [128, NB], I32, kind="ExternalInput")
    eim_full = nc.dram_tensor("eim_full", [NUM_ENTITIES, DIM], F32, kind="ExternalInput")
    ere_full = nc.dram_tensor("ere_full", [NUM_ENTITIES, DIM], F32, kind="ExternalInput")
    eimT = nc.dram_tensor("eimT", [DIM, ESH], F32, kind="ExternalInput")
    ereT = nc.dram_tensor("ereT", [DIM, ESH], F32, kind="ExternalInput")
    rim = nc.dram_tensor("rim", [NUM_REL, DIM], F32, kind="ExternalInput")
    rre = nc.dram_tensor("rre", [NUM_REL, DIM], F32, kind="ExternalInput")
    maskrep = nc.dram_tensor("maskrep", [128, ESH], F32, kind="ExternalInput")
    out = nc.dram_tensor("out", [BATCH, ESH], F32, kind="ExternalOutput")

    mm_dt = F32R if use_f32r else F32

    with tile.TileContext(nc) as tc:
        with (
            tc.tile_pool(name="cpool", bufs=1) as cpool,
            tc.tile_pool(name="gpool", bufs=3) as gpool,
            tc.tile_pool(name="epool", bufs=3) as epool,
            tc.tile_pool(name="persist", bufs=1) as ppool,
            tc.tile_pool(name="tps", bufs=4, space="PSUM") as tpsum,
            tc.tile_pool(name="rhspool", bufs=2) as rhspool,
            tc.tile_pool(name="mpool", bufs=2) as mpool,
            tc.tile_pool(name="outpool", bufs=4) as outpool,
            tc.tile_pool(name="psum", bufs=4, space="PSUM") as psum,
        ):
          for _rep in range(nrep):
            # ---- constants / index preprocessing (on device) ----
            identity = cpool.tile([128, 128], F32)
            make_identity(nc, identity[:])

            hix_sb = cpool.tile([128, NB], I32)
            nc.sync.dma_start(hix_sb[:], hix[:])
            rix_sb = cpool.tile([128, NB], I32)
            nc.sync.dma_start(rix_sb[:], rix[:])

            rf = cpool.tile([128, NB], F32)
            nc.vector.tensor_copy(rf[:], rix_sb[:])
            ge = cpool.tile([128, NB], F32)
            nc.vector.tensor_scalar(
                ge[:], rf[:], float(NUM_REL) - 0.5, None, op0=mybir.AluOpType.is_gt
            )
            # sign s = 1 - 2*[r >= NUM_REL]
            sall = cpool.tile([128, NB], F32)
            nc.vector.tensor_scalar(
                sall[:], ge[:], -2.0, 1.0,
                op0=mybir.AluOpType.mult, op1=mybir.AluOpType.add,
            )
            # r_eff = r - NUM_REL*[r >= NUM_REL]
            ge500 = cpool.tile([128, NB], F32)
            nc.vector.tensor_scalar(
                ge500[:], ge[:], float(NUM_REL), None, op0=mybir.AluOpType.mult
            )
            reff_f = cpool.tile([128, NB], F32)
            nc.vector.tensor_sub(reff_f[:], rf[:], ge500[:])
            reff = cpool.tile([128, NB], I32)
            nc.vector.tensor_copy(reff[:], reff_f[:])

            # ---- gather + elementwise + transpose: build tmp1T/tmp2T ----
            # tmp{1,2}T layout: [128 (d within chunk), NB*DIM] where column
            # bt*DIM + c*128 + j holds tmp[bt*128 + j, c*128 + d]
            tmp1T = [ppool.tile([128, DIM], mm_dt, tag=f"t1T{b}", name=f"t1T{b}") for b in range(NB)]
            tmp2T = [ppool.tile([128, DIM], mm_dt, tag=f"t2T{b}", name=f"t2T{b}") for b in range(NB)]

            if not do_prologue:
                scratch = cpool.tile([128, DIM], F32)
                nc.vector.memset(scratch[:], 0.001)
                for b in range(NB):
                    nc.vector.tensor_copy(tmp1T[b][:], scratch[:])
                    nc.vector.tensor_copy(tmp2T[b][:], scratch[:])
            for bt in range(NB if do_prologue else 0):
                h_im = gpool.tile([128, DIM], F32, tag="h_im")
                nc.gpsimd.indirect_dma_start(
                    out=h_im[:], out_offset=None, in_=eim_full[:],
                    in_offset=IndirectOffsetOnAxis(ap=hix_sb[:, bt : bt + 1], axis=0),
                )
                h_re = gpool.tile([128, DIM], F32, tag="h_re")
                nc.gpsimd.indirect_dma_start(
                    out=h_re[:], out_offset=None, in_=ere_full[:],
                    in_offset=IndirectOffsetOnAxis(ap=hix_sb[:, bt : bt + 1], axis=0),
                )
                r_im = gpool.tile([128, DIM], F32, tag="r_im")
                nc.gpsimd.indirect_dma_start(
                    out=r_im[:], out_offset=None, in_=rim[:],
                    in_offset=IndirectOffsetOnAxis(ap=reff[:, bt : bt + 1], axis=0),
                )
                r_re = gpool.tile([128, DIM], F32, tag="r_re")
                nc.gpsimd.indirect_dma_start(
                    out=r_re[:], out_offset=None, in_=rre[:],
                    in_offset=IndirectOffsetOnAxis(ap=reff[:, bt : bt + 1], axis=0),
                )

                # r_im' = s * r_im  (per-partition scalar)
                rimp = epool.tile([128, DIM], F32, tag="rimp")
                nc.vector.tensor_scalar(
                    rimp[:], r_im[:], sall[:, bt : bt + 1], None,
                    op0=mybir.AluOpType.mult,
                )
                # tmp1 = h_im*r_re + h_re*r_im'
                pa = epool.tile([128, DIM], F32, tag="pa")
                nc.vector.tensor_mul(pa[:], h_im[:], r_re[:])
                pb = epool.tile([128, DIM], F32, tag="pb")
                nc.vector.tensor_mul(pb[:], h_re[:], rimp[:])
                tmp1 = epool.tile([128, DIM], F32, tag="tmp1")
                nc.vector.tensor_add(tmp1[:], pa[:], pb[:])
                # tmp2 = h_re*r_re - h_im*r_im'
                pc = epool.tile([128, DIM], F32, tag="pc")
                nc.vector.tensor_mul(pc[:], h_re[:], r_re[:])
                pd = epool.tile([128, DIM], F32, tag="pd")
                nc.vector.tensor_mul(pd[:], h_im[:], rimp[:])
                tmp2 = epool.tile([128, DIM], F32, tag="tmp2")
                nc.vector.tensor_sub(tmp2[:], pc[:], pd[:])

                for src, dst in ((tmp1, tmp1T[bt]), (tmp2, tmp2T[bt])):
                    for c in range(NC_D):
                        pt = tpsum.tile([128, 128], F32, tag="pt")
                        nc.tensor.transpose(
                            pt[:], src[:, c * 128 : (c + 1) * 128], identity[:]
                        )
                        nc.vector.tensor_copy(
                            dst[:, c * 128 : (c + 1) * 128], pt[:]
                        )

            # ---- main GEMM: out[b, e] = tmp1 @ E_im^T + tmp2 @ E_re^T + mask ----
            if not do_rhs_dma:
                rhs_static = rhspool.tile([128, 2 * NC_D * ET], mm_dt, tag="rhss")
                scratch2 = cpool.tile([128, 2 * NC_D * ET], F32)
                nc.vector.memset(scratch2[:], 0.001)
                nc.vector.tensor_copy(rhs_static[:], scratch2[:])
            for et in range(N_ET):
                e0 = et * ET
                if do_rhs_dma:
                    rhs = rhspool.tile([128, 2 * NC_D * ET], mm_dt, tag="rhs")
                    for t, eT in enumerate((eimT, ereT)):
                        for c in range(NC_D):
                            # SWDGE (gpsimd) casts f32 -> f32r during the DMA;
                            # plain HWDGE path when matmuls run in plain f32.
                            dma_eng = nc.gpsimd if use_f32r else nc.sync
                            dma_eng.dma_start(
                                rhs[:, (t * NC_D + c) * ET : (t * NC_D + c + 1) * ET],
                                eT[c * 128 : (c + 1) * 128, e0 : e0 + ET],
                            )
                else:
                    rhs = rhs_static
                mtile = mpool.tile([128, ET], F32, tag="mtile")
                nc.sync.dma_start(mtile[:], maskrep[:, e0 : e0 + ET])

                for bt in range(NB):
                    store = store_all or et == 0
                    if not do_mm:
                        continue
                    ps = psum.tile([128, ET], F32, tag="ps")
                    k = 0
                    for t, tT in enumerate((tmp1T, tmp2T)):
                        for c in range(NC_D):
                            nc.tensor.matmul(
                                ps[:],
                                lhsT=tT[bt][:, c * 128 : (c + 1) * 128],
                                rhs=rhs[
                                    :, (t * NC_D + c) * ET : (t * NC_D + c + 1) * ET
                                ],
                                start=(k == 0),
                                stop=(k == 2 * NC_D - 1),
                            )
                            k += 1
                    if store:
                        ot = outpool.tile([128, ET], F32, tag="ot")
                        nc.vector.tensor_add(ot[:], ps[:], mtile[:])
                        nc.sync.dma_start(
                            out[bt * 128 : (bt + 1) * 128, e0 : e0 + ET], ot[:]
                        )

    nc.compile()
    return nc


_NC_CACHE = {}


def _get_module(use_f32r=True):
    key = use_f32r
    if key not in _NC_CACHE:
        _NC_CACHE[key] = build_module(use_f32r)
    return _NC_CACHE[key]


def make_in_maps(h, r, E_im, E_re, R_im, R_re, mask):
    """Host-side sharding / layout prep (value-independent transforms only)."""
    h32 = np.ascontiguousarray(np.asarray(h, dtype=np.int32).reshape(NB, 128).T)
    r32 = np.ascontiguousarray(np.asarray(r, dtype=np.int32).reshape(NB, 128).T)
    E_im = np.asarray(E_im, dtype=np.float32)
    E_re = np.asarray(E_re, dtype=np.float32)
    rim = np.ascontiguousarray(np.asarray(R_im, dtype=np.float32)[:NUM_REL])
    rre = np.ascontiguousarray(np.asarray(R_re, dtype=np.float32)[:NUM_REL])
    mask = np.asarray(mask, dtype=np.float32).reshape(1, NUM_ENTITIES)

    in_maps = []
    for k in range(N_CORES):
        sl = slice(k * ESH, (k + 1) * ESH)
        in_maps.append(
            {
                "hix": h32,
                "rix": r32,
                "eim_full": E_im,
                "ere_full": E_re,
                "eimT": np.ascontiguousarray(E_im[sl].T),
                "ereT": np.ascontiguousarray(E_re[sl].T),
                "rim": rim,
                "rre": rre,
                "maskrep": np.ascontiguousarray(
                    np.broadcast_to(mask[:, sl], (128, ESH))
                ),
            }
        )
    return in_maps


def kernel(h, r, E_im, E_re, R_im, R_re, mask):
    nc = _get_module()
    in_maps = make_in_maps(h, r, E_im, E_re, R_im, R_re, mask)
    res = run_bass_kernel_spmd(nc, in_maps, core_ids=list(range(N_CORES)))
    return np.concatenate([res.results[k]["out"] for k in range(N_CORES)], axis=1)



# revision 14
# speedup vs baseline: 1.4828x; 1.4828x over previous
"""ComplEx scoring kernel for Trainium2, sharded over 8 NeuronCores.

Computes: result[b, e] = tmp1[b] . E_im[e] + tmp2[b] . E_re[e] + mask[e]
where tmp1/tmp2 are complex-product combinations of gathered entity and
relation embeddings (with inverse-relation sign handling).

Sharding: entity dimension (100000) split across 8 cores (12500 each).
Batch and relation tables replicated. Each core redundantly computes the
gathered tmp1/tmp2 (needs the full entity table for the gather), then
GEMMs against its own entity shard and emits [1024, 12500] logits.

Performance structure (per core, per exec):
  - PE roofline: 1600 accumulating matmuls [128x128]@[128x500] (bf16,
    1 cycle/row) ~= 333 us. The PE instruction stream is PURE matmuls:
    the tmp transposes go through the HWDGE xbar (dma_start_transpose
    on the Activation queue), not the PE.
  - The inverse-relation sign is folded into host-side sign-extended
    relation tables (rows 500..999 = -R_im / +R_re), so there is no
    on-device index math at all: h and r are raw gather indices.
  - Prologue = 4 wide indirect gathers (bf16) + 6 wide elementwise ops
    (engine-assigned by the scheduler) + 64 xbar transposes.
  - rhs weights pre-tiled + pre-cast to bf16 on the host into ONE
    contiguous DRAM block per e-tile (8 weight chunks + mask columns):
    a single ~1.1 MB HWDGE DMA per e-tile.
  - All prologue buffers are at least double-buffered so that in a
    chained (nrep>1) execution, rep i+1's prologue fully overlaps rep
    i's matmul stream -> steady-state per-rep cost ~= PE busy time.
"""

import sys

sys.path.insert(0, "/opt/trn_rl_repo")

import numpy as np
import ml_dtypes

import concourse.bacc as bacc
import concourse.bass as bass
import concourse.mybir as mybir
import concourse.tile as tile
from concourse.bass import IndirectOffsetOnAxis
from concourse.bass_utils import run_bass_kernel_spmd

F32 = mybir.dt.float32
BF16 = mybir.dt.bfloat16
I32 = mybir.dt.int32

NUM_ENTITIES = 100000
DIM = 512
BATCH = 1024
NUM_REL = 500   # NUM_REL_TOTAL // 2
NUM_REL_TOTAL = 1000
N_CORES = 8
ESH = NUM_ENTITIES // N_CORES  # 12500 entities per core
ET = 500                       # entity tile (free dim of each matmul)
N_ET = ESH // ET               # 25 e-tiles per core
NB = BATCH // 128              # 8 batch tiles
NC_D = DIM // 128              # 4 contraction chunks per table
KB = 2 * NC_D                  # 8 weight blocks (2 tables x 4 chunks)
FREE = KB * ET + ET            # rhs tile free dim: weights + mask block
WIDE = NB * DIM                # 4096: all-batch wide tiles


def build_module(
    use_f32r=False,     # kept for test.py compat; ignored (bf16 path only)
    nrep=1,
    pe_transpose=False, # fallback: transpose tmp on the PE instead of xbar
    do_prologue=True,
    do_rhs_dma=True,
    do_mm=True,
    store_all=True,
):
    del use_f32r
    nc = bacc.Bacc("TRN2", target_bir_lowering=False, debug=False)

    hix = nc.dram_tensor("hix", [128, NB], I32, kind="ExternalInput")
    rix = nc.dram_tensor("rix", [128, NB], I32, kind="ExternalInput")
    eimb = nc.dram_tensor("eimb", [NUM_ENTITIES, DIM], BF16, kind="ExternalInput")
    ereb = nc.dram_tensor("ereb", [NUM_ENTITIES, DIM], BF16, kind="ExternalInput")
    # sign-extended relation tables: rows 500..999 hold -R_im / +R_re
    rimx = nc.dram_tensor("rimx", [NUM_REL_TOTAL, DIM], BF16, kind="ExternalInput")
    rrex = nc.dram_tensor("rrex", [NUM_REL_TOTAL, DIM], BF16, kind="ExternalInput")
    # pre-tiled weights+mask: rows et*128..(et+1)*128 hold e-tile et's
    # [128, FREE] rhs tile (8 weight chunks of ET cols, then ET mask cols)
    rhs_all = nc.dram_tensor("rhs_all", [N_ET * 128, FREE], BF16, kind="ExternalInput")
    out = nc.dram_tensor("out", [BATCH, ESH], F32, kind="ExternalOutput")

    with tile.TileContext(nc) as tc:
        with (
            tc.tile_pool(name="cpool", bufs=2) as cpool,
            tc.tile_pool(name="gpool", bufs=1) as gpool,
            tc.tile_pool(name="epool", bufs=1) as epool,
            tc.tile_pool(name="persist", bufs=2) as ppool,
            tc.tile_pool(name="rhspool", bufs=3) as rhspool,
            tc.tile_pool(name="outpool", bufs=6) as outpool,
            tc.tile_pool(name="psum", bufs=8, space="PSUM") as psum,
        ):
          for _rep in range(nrep):
            # All loads go through the Pool/SWDGE queue; SP carries only the
            # output stores and ACT only the xbar transposes. This keeps rep
            # i+1's prologue from queueing behind rep i's stores (SP is paced
            # by the matmul stream), so consecutive reps fully overlap.
            hix_sb = cpool.tile([128, NB], I32, tag="hix")
            nc.gpsimd.dma_start(hix_sb[:], hix[:])
            rix_sb = cpool.tile([128, NB], I32, tag="rix")
            nc.gpsimd.dma_start(rix_sb[:], rix[:])

            # ---- wide gathers: one indirect DMA per table, all batch ----
            # tile[p, j*512+d] = T[idx[j*128+p], d]
            himw = gpool.tile([128, WIDE], BF16, tag="himw")
            hrew = gpool.tile([128, WIDE], BF16, tag="hrew")
            rimw = gpool.tile([128, WIDE], BF16, tag="rimw")
            rrew = gpool.tile([128, WIDE], BF16, tag="rrew")
            if do_prologue:
                # HW SWDGE uses only ONE offset per partition per indirect
                # DMA (multi-column offset APs gather consecutive rows
                # instead), so gather one [128, DIM] block per batch tile.
                for bt in range(NB):
                    for buf, tab, ix in (
                        (himw, eimb, hix_sb),
                        (hrew, ereb, hix_sb),
                        (rrew, rrex, rix_sb),
                        (rimw, rimx, rix_sb),
                    ):
                        nc.gpsimd.indirect_dma_start(
                            out=buf[:, bt * DIM : (bt + 1) * DIM],
                            out_offset=None, in_=tab[:],
                            in_offset=IndirectOffsetOnAxis(
                                ap=ix[:, bt : bt + 1], axis=0
                            ),
                        )

                # ---- wide elementwise (sign already in rimx):
                #   tmp1 = h_im*r_re + h_re*r_im'
                #   tmp2 = h_re*r_re - h_im*r_im'
                m1 = epool.tile([128, WIDE], BF16, tag="m1")
                nc.any.tensor_mul(m1[:], himw[:], rrew[:])
                m2 = epool.tile([128, WIDE], BF16, tag="m2")
                nc.any.tensor_mul(m2[:], hrew[:], rimw[:])
                tmp1w = epool.tile([128, WIDE], BF16, tag="t1w")
                nc.any.tensor_add(tmp1w[:], m1[:], m2[:])
                m3 = epool.tile([128, WIDE], BF16, tag="m3")
                nc.any.tensor_mul(m3[:], hrew[:], rrew[:])
                m4 = epool.tile([128, WIDE], BF16, tag="m4")
                nc.any.tensor_mul(m4[:], himw[:], rimw[:])
                tmp2w = epool.tile([128, WIDE], BF16, tag="t2w")
                nc.any.tensor_sub(tmp2w[:], m3[:], m4[:])
            else:
                tmp1w = epool.tile([128, WIDE], BF16, tag="t1w")
                nc.vector.memset(tmp1w[:], 0.001)
                tmp2w = epool.tile([128, WIDE], BF16, tag="t2w")
                nc.vector.memset(tmp2w[:], 0.001)

            # ---- transposes: one wide xbar DMA-transpose per tensor.
            # out viewed [128, 32, 128]: out[p, q, j] = in[j, q*128+p], i.e.
            # column block q = bt*NC_D+c holds tmp[bt*128+j, c*128+p] -- the
            # lhsT chunk for (bt, c) is tT[:, (bt*NC_D+c)*128 : +128].
            tmp1T = ppool.tile([128, WIDE], BF16, tag="t1T", name="t1T")
            tmp2T = ppool.tile([128, WIDE], BF16, tag="t2T", name="t2T")
            for src, dst in ((tmp1w, tmp1T), (tmp2w, tmp2T)):
                nc.scalar.dma_start_transpose(
                    dst[:].rearrange("p (q j) -> p q j", j=128), src[:]
                )

            # ---- main GEMM: out[b, e] = tmp1 @ E_im^T + tmp2 @ E_re^T + mask ----
            if not do_rhs_dma:
                rhs_static = rhspool.tile([128, FREE], BF16, tag="rhss")
                nc.vector.memset(rhs_static[:], 0.001)
            for et in range(N_ET):
                e0 = et * ET
                if do_rhs_dma:
                    rhs = rhspool.tile([128, FREE], BF16, tag="rhs")
                    # ACT queue (with the transposes): stores pace with the
                    # matmul stream on SP, so rhs prefetch must not sit
                    # behind them
                    nc.scalar.dma_start(
                        rhs[:], rhs_all[et * 128 : (et + 1) * 128, :]
                    )
                else:
                    rhs = rhs_static

                for bt in range(NB):
                    store = store_all or et == 0
                    if not do_mm:
                        continue
                    ps = psum.tile([128, ET], F32, tag="ps")
                    k = 0
                    for tT in (tmp1T, tmp2T):
                        for c in range(NC_D):
                            q = bt * NC_D + c
                            nc.tensor.matmul(
                                ps[:],
                                lhsT=tT[:, q * 128 : (q + 1) * 128],
                                rhs=rhs[:, k * ET : (k + 1) * ET],
                                start=(k == 0),
                                stop=(k == KB - 1),
                            )
                            k += 1
                    if store:
                        ot = outpool.tile([128, ET], F32, tag="ot")
                        # psum + mask (mask lives in the tail block of rhs)
                        nc.any.tensor_add(
                            ot[:], ps[:], rhs[:, KB * ET : KB * ET + ET]
                        )
                        nc.sync.dma_start(
                            out[bt * 128 : (bt + 1) * 128, e0 : e0 + ET], ot[:]
                        )

    nc.compile()
    return nc


_NC_CACHE = {}


def _get_module(use_f32r=False):
    key = use_f32r
    if key not in _NC_CACHE:
        _NC_CACHE[key] = build_module(use_f32r)
    return _NC_CACHE[key]


def make_in_maps(h, r, E_im, E_re, R_im, R_re, mask, use_f32r=False):
    """Host-side sharding / layout prep (value-independent transforms only)."""
    del use_f32r
    bf16 = ml_dtypes.bfloat16
    h32 = np.ascontiguousarray(np.asarray(h, dtype=np.int32).reshape(NB, 128).T)
    r32 = np.ascontiguousarray(np.asarray(r, dtype=np.int32).reshape(NB, 128).T)
    E_im = np.asarray(E_im, dtype=np.float32)
    E_re = np.asarray(E_re, dtype=np.float32)
    eimb = np.ascontiguousarray(E_im.astype(bf16))
    ereb = np.ascontiguousarray(E_re.astype(bf16))
    rim = np.asarray(R_im, dtype=np.float32)[:NUM_REL]
    rre = np.asarray(R_re, dtype=np.float32)[:NUM_REL]
    # sign-extended relation tables: r>=500 selects the inverse relation,
    # which negates the imaginary part (and keeps the real part)
    rimx = np.ascontiguousarray(
        np.concatenate([rim, -rim], axis=0).astype(bf16)
    )
    rrex = np.ascontiguousarray(
        np.concatenate([rre, rre], axis=0).astype(bf16)
    )
    mask = np.asarray(mask, dtype=np.float32).reshape(1, NUM_ENTITIES)

    in_maps = []
    for k in range(N_CORES):
        sl = slice(k * ESH, (k + 1) * ESH)
        # weights: [c*128+p, et*500+e] -> [et, p, j=t*4+c, e] -> [et*128, 8*500]
        A = E_im[sl].T.reshape(NC_D, 128, N_ET, ET)
        B = E_re[sl].T.reshape(NC_D, 128, N_ET, ET)
        W = np.concatenate([A, B], axis=0).transpose(2, 1, 0, 3)
        W = W.reshape(N_ET * 128, KB * ET)
        M = np.broadcast_to(
            mask[0, sl].reshape(N_ET, 1, ET), (N_ET, 128, ET)
        ).reshape(N_ET * 128, ET)
        rhs_host = np.ascontiguousarray(
            np.concatenate([W, M], axis=1).astype(bf16)
        )
        in_maps.append(
            {
                "hix": h32,
                "rix": r32,
                "eimb": eimb,
                "ereb": ereb,
                "rimx": rimx,
                "rrex": rrex,
                "rhs_all": rhs_host,
            }
        )
    return in_maps


def kernel(h, r, E_im, E_re, R_im, R_re, mask):
    nc = _get_module()
    in_maps = make_in_maps(h, r, E_im, E_re, R_im, R_re, mask)
    res = run_bass_kernel_spmd(nc, in_maps, core_ids=list(range(N_CORES)))
    return np.concatenate([res.results[k]["out"] for k in range(N_CORES)], axis=1)
